# revision 1
# baseline (speedup 1.0000x reference)
"""Trainium2 Bass kernel for nn_DressedQuantumNet.

Math reformulation (exact, up to float rounding):
  pre_out = x @ pre_w.T + pre_b                  # [B,4]
  theta_w = (pi/4)*tanh(pre_out_w) + pi/4        # in (0, pi/2)
  v_w     = [cos theta_w, sin theta_w]           # per-qubit state (positive)
  psi     = v_0 (x) v_1 (x) v_2 (x) v_3          # [B,16] product state
  phi     = M @ psi        # M = fixed 16x16 matrix of the CNOT/RY circuit
  out     = (phi*phi)^T P + post_b  # P[i,c] = sum_w post_w[c,w] * z_w(i)

Device strategy (pure data parallel over 8 cores, 8192 samples each):
  - x is downcast to bf16 on host (halves HBM traffic; fp32 accumulation in
    PSUM keeps the matmul accurate).
  - x tiles are loaded transposed via the DMA xbar (dma transpose), so the
    contraction dim (D=512, in 4 chunks of 128) lands on SBUF partitions.
  - pre-matmul: lhsT = xT chunk [128d, 128b], rhs = pre_w^T chunk [128d, 4]
    accumulated over the 4 chunks into PSUM [128b, 4].
  - bias + PSUM->SBUF handled by one vector add with a broadcast bias AP.
  - angles/trig on ScalarE (Tanh + 2x Sin with scale/bias folding cos).
  - psi built with 3 broadcast-AP vector multiplies.
  - quantum circuit: PE transpose of psi -> [16 comps x 8 tiles, 128 samples],
    then two block-diagonal matmuls (M and P) on the tensor engine.
"""

import os
import sys

for _p in ("/opt/trn_rl_repo",):
    if os.path.isdir(_p) and _p not in sys.path:
        sys.path.insert(0, _p)

import math
import numpy as np
import ml_dtypes
from contextlib import ExitStack

import concourse.bass as bass
import concourse.bacc as bacc
import concourse.mybir as mybir
from concourse.tile import TileContext, add_dep_helper
from concourse.bass_utils import run_bass_kernel_spmd

F32 = mybir.dt.float32
BF16 = mybir.dt.bfloat16
AF = mybir.ActivationFunctionType
PI4 = math.pi / 4.0

N_CORES = 8
B_FULL, D, C = 65536, 512, 10
B = B_FULL // N_CORES          # 8192 samples per core
N_QUBITS, Q_DEPTH = 4, 6
TILES = B // 128               # 64 sample tiles of 128
GROUPS = 16                    # phase-1 groups of 512 samples (4 tiles)
CHUNKS = 4                     # phase-2 chunks of 2048 samples (16 tiles)
G_PER_C = GROUPS // CHUNKS


# ---------------------------------------------------------------- host math
def _apply_1q(state, gate, wire):
    state = np.moveaxis(state, wire, 0)
    state = np.tensordot(gate, state, axes=((1,), (0,)))
    return np.moveaxis(state, 0, wire)


def _apply_cnot(state, ctrl, tgt):
    state = np.moveaxis(state, (ctrl, tgt), (0, 1))
    state = np.stack([state[0], state[1][::-1]], axis=0)
    return np.moveaxis(state, (0, 1), (ctrl, tgt))


def _ry(theta):
    c, s = np.cos(theta * 0.5), np.sin(theta * 0.5)
    return np.array([[c, -s], [s, c]])


def _build_M(q_params: np.ndarray) -> np.ndarray:
    """16x16 matrix of the fixed part of the circuit (after the per-sample
    RY layer): 6 repetitions of [CNOT(0,1), CNOT(2,3), CNOT(1,2), RY layer]."""
    qw = np.asarray(q_params, np.float64).reshape(Q_DEPTH, N_QUBITS)
    M = np.zeros((16, 16), np.float64)
    for i in range(16):
        state = np.zeros(16, np.float64)
        state[i] = 1.0
        state = state.reshape((2,) * N_QUBITS)
        for k in range(Q_DEPTH):
            for a in range(0, N_QUBITS - 1, 2):
                state = _apply_cnot(state, a, a + 1)
            for a in range(1, N_QUBITS - 1, 2):
                state = _apply_cnot(state, a, a + 1)
            for w in range(N_QUBITS):
                state = _apply_1q(state, _ry(qw[k, w]), w)
        M[:, i] = state.reshape(16)
    return M


def _build_P(post_w: np.ndarray) -> np.ndarray:
    """P[i, c] = sum_w post_w[c, w] * z_w(i), where z_w(i) flips sign with
    bit (3-w) of the state index i (axis 0 of the state = qubit 0)."""
    post_w = np.asarray(post_w, np.float64)
    i = np.arange(16)
    z = np.stack([1.0 - 2.0 * ((i >> (3 - w)) & 1) for w in range(N_QUBITS)], 1)
    return z @ post_w.T  # [16, 10]


# ---------------------------------------------------------------- bass build
def build_nc(sim_compat: bool = False) -> bass.Bass:
    # Bacc (not raw Bass): its finalize() runs generate_event_semaphores,
    # which splits multi-semaphore waits to satisfy the TRN2 one-wait-per-
    # instruction ISA limit.
    nc = bacc.Bacc(None)
    x = nc.dram_tensor("x", [B, D], BF16, kind="ExternalInput")
    pre_wt = nc.dram_tensor("pre_wt", [128, 16], BF16, kind="ExternalInput")
    pre_b41 = nc.dram_tensor("pre_b41", [4, 1], F32, kind="ExternalInput")
    mbd = nc.dram_tensor("mbd", [128, 128], F32, kind="ExternalInput")
    pbd = nc.dram_tensor("pbd", [128, 80], F32, kind="ExternalInput")
    post_b80 = nc.dram_tensor("post_b80", [80, 1], F32, kind="ExternalInput")
    trigb = nc.dram_tensor("trigb", [128, 2], F32, kind="ExternalInput")
    ident = nc.dram_tensor("ident", [128, 128], F32, kind="ExternalInput")
    # transposed on device: out[tile, class, sample-in-tile]; host flips back
    out = nc.dram_tensor("out", [TILES, C, 128], F32, kind="ExternalOutput")

    with ExitStack() as ctx:
        tc = ctx.enter_context(TileContext(nc))
        consts = ctx.enter_context(tc.tile_pool(name="consts", bufs=1))
        # all 64 xT tiles stay resident (8 MB) — avoids WAR waits on the
        # transpose DMAs (DmaTransposeAnt supports very few sync waits)
        xt_pool = ctx.enter_context(tc.tile_pool(name="xt", bufs=GROUPS))
        work = ctx.enter_context(tc.tile_pool(name="work", bufs=2))
        ps_po = ctx.enter_context(tc.tile_pool(name="ps_po", space="PSUM", bufs=2))
        ps2 = ctx.enter_context(tc.tile_pool(name="ps2", space="PSUM", bufs=4))

        pre_wt_sb = consts.tile([128, 16], BF16)
        nc.scalar.dma_start(pre_wt_sb, pre_wt[:, :])
        pre_b_sb = consts.tile([4, 1], F32)
        nc.scalar.dma_start(pre_b_sb, pre_b41[:, :])
        mbd_sb = consts.tile([128, 128], F32)
        nc.scalar.dma_start(mbd_sb, mbd[:, :])
        pbd_sb = consts.tile([128, 80], F32)
        nc.scalar.dma_start(pbd_sb, pbd[:, :])
        pb80_sb = consts.tile([80, 1], F32)
        nc.scalar.dma_start(pb80_sb, post_b80[:, :])
        trigb_sb = consts.tile([128, 2], F32)
        nc.scalar.dma_start(trigb_sb, trigb[:, :])
        id_sb = consts.tile([128, 128], F32)
        last_const = nc.scalar.dma_start(id_sb, ident[:, :])

        out2_sb = consts.tile([80, 128 * 2 * CHUNKS], F32)  # [80, 1024]
        # feature-row tanh staging, bf16, padded to 16 partitions so the
        # SBUF->SBUF xbar transpose (rows%16==0) can flip it to sample-major;
        # rows 4..15 are never written or consumed
        t16_sb = consts.tile([16, B], BF16)  # [16, 8192]
        nc.gpsimd.memset(t16_sb[:, :], 0.0)  # init the padding rows once

        # pin the activation table to silu_and_others once: it contains
        # silu+tanh+sin+square+identity, so no further table loads happen.
        # (CoreSim can't evaluate Silu; the sim build substitutes Tanh —
        # the value is unused either way.)
        silu_sb = consts.tile([128, 1], F32)
        nc.scalar.activation(silu_sb, trigb_sb[:, 0:1],
                             AF.Tanh if sim_compat else AF.Silu)

        for cki in range(CHUNKS):
            # ---- phase 1: pre-net for this chunk's 4 groups of 512 samples
            for gi in range(G_PER_C):
                g = cki * G_PER_C + gi
                # one xbar transpose per 512-sample group with a fully
                # contiguous 512KB DRAM source: out[p, k, b] = x[b, 128k+p];
                # alternate between the two HWDGE queues (SP / ACT)
                xt = xt_pool.tile([128, 4 * 512], BF16, name="xt", tag="xt")
                # NOTE: all xbar transposes must stay on ONE HWDGE queue —
                # concurrent transpose streams on the SP and ACT rings
                # corrupt data through the shared xbar (measured twice).
                xpose = nc.sync.dma_start(
                    xt[:, :].rearrange("p (k b) -> p k b", k=4),
                    x[512 * g:512 * (g + 1), :],
                    transpose=True)
                # keep all plain copies scheduled before all xbar transposes
                # (every copy<->transpose transition serializes the DMA ring)
                add_dep_helper(last_const.ins, xpose.ins, sync=False,
                               reason="consts before xbar transposes")
                # pre-out transposed: lhsT is the tiny pre_w chunk (4-column
                # LDWEIGHTS), xT streams as the moving operand at 1 col/cycle
                po = ps_po.tile([4, 512], F32, name="po", tag="po")
                for k in range(4):
                    nc.tensor.matmul(
                        po[:, :],
                        lhsT=pre_wt_sb[:, 4 * k:4 * (k + 1)],
                        rhs=xt[:, 512 * k:512 * (k + 1)],
                        start=(k == 0), stop=(k == 3))
                # fused bias + tanh, straight out of PSUM, bf16 out
                nc.scalar.activation(t16_sb[0:4, 512 * g:512 * (g + 1)], po,
                                     AF.Tanh, bias=pre_b_sb[:, :])

            # ---- phase 2: trig + quantum net for this chunk (16 tiles)
            # back to sample-major layout with one tiny xbar transpose (same
            # DMA mode as the x loads, so no ring-mode transitions):
            # th_bf[p, t, i] = t16[i, 2048*cki + 128t + p]
            th_bf = work.tile([128, 256], BF16, name="th_bf", tag="th_bf")
            nc.sync.dma_start(
                th_bf[:, :].rearrange("p (t i) -> p t i", i=16),
                t16_sb[:, 2048 * cki:2048 * (cki + 1)],
                transpose=True)
            cs = work.tile([128, 128], F32, name="cs", tag="cs")
            cs4 = cs[:, :].rearrange("p (t w x) -> p t w x", w=4, x=2)
            th3 = th_bf[:, :].rearrange("p (t i) -> p t i", i=16)[:, :, 0:4]
            # cos(theta) = sin(pi/4*tanh + 3pi/4); sin(theta) = sin(.. + pi/4)
            nc.scalar.activation(cs4[:, :, :, 0], th3, AF.Sin,
                                 bias=trigb_sb[:, 0:1], scale=PI4)
            nc.scalar.activation(cs4[:, :, :, 1], th3, AF.Sin,
                                 bias=trigb_sb[:, 1:2], scale=PI4)
            cs8 = cs[:, :].rearrange("p (t w) -> p t w", w=8)
            v01 = work.tile([128, 64], F32, name="v01", tag="v01")
            v23 = work.tile([128, 64], F32, name="v23", tag="v23")
            nc.vector.tensor_tensor(
                out=v01[:, :].rearrange("p (t a b) -> p t a b", a=2, b=2),
                in0=cs8[:, :, 0:2].unsqueeze(3).broadcast_to((128, 16, 2, 2)),
                in1=cs8[:, :, 2:4].unsqueeze(2).broadcast_to((128, 16, 2, 2)),
                op=mybir.AluOpType.mult)
            nc.vector.tensor_tensor(
                out=v23[:, :].rearrange("p (t a b) -> p t a b", a=2, b=2),
                in0=cs8[:, :, 4:6].unsqueeze(3).broadcast_to((128, 16, 2, 2)),
                in1=cs8[:, :, 6:8].unsqueeze(2).broadcast_to((128, 16, 2, 2)),
                op=mybir.AluOpType.mult)
            psi = work.tile([128, 256], F32, name="psi", tag="psi")
            nc.vector.tensor_tensor(
                out=psi[:, :].rearrange("p (t a b) -> p t a b", a=4, b=4),
                in0=v01[:, :].rearrange("p (t i) -> p t i", i=4)
                    .unsqueeze(3).broadcast_to((128, 16, 4, 4)),
                in1=v23[:, :].rearrange("p (t i) -> p t i", i=4)
                    .unsqueeze(2).broadcast_to((128, 16, 4, 4)),
                op=mybir.AluOpType.mult)

            for h in range(2):
                psiT_ps = ps2.tile([128, 128], F32, name="psiT_ps", tag="p2")
                nc.tensor.transpose(
                    psiT_ps, psi[:, 128 * h:128 * (h + 1)], id_sb[:, :])
                psiT = work.tile([128, 128], F32, name="psiT", tag="psiT")
                nc.vector.tensor_copy(psiT, psiT_ps)
                phiT_ps = ps2.tile([128, 128], F32, name="phiT_ps", tag="p2")
                nc.tensor.matmul(phiT_ps, lhsT=mbd_sb[:, :], rhs=psiT,
                                 start=True, stop=True)
                phi2 = work.tile([128, 128], F32, name="phi2", tag="phi2")
                nc.scalar.activation(phi2, phiT_ps, AF.Square)
                o10_ps = ps2.tile([80, 128], F32, name="o10_ps", tag="p2")
                nc.tensor.matmul(o10_ps, lhsT=pbd_sb[:, :], rhs=phi2,
                                 start=True, stop=True)
                # bias-add lands directly in the transposed output staging
                # tile; stored once at the end (keeps 512B-contiguous DMA
                # descriptors and no copy<->transpose ring transitions)
                hh = 2 * cki + h
                nc.scalar.activation(out2_sb[:, 128 * hh:128 * (hh + 1)],
                                     o10_ps, AF.Identity, bias=pb80_sb[:, :])

        # single store at the very end, in transposed layout [64, 10, 128];
        # the host flips it back to [8192, 10]
        nc.scalar.dma_start(
            out[:, :, :].rearrange("(h t) c p -> (t c) h p", h=8),
            out2_sb[:, :].rearrange("p (h b) -> p h b", h=8))

    nc.finalize()  # bacc: register alloc + event-semaphore wait splitting
    return nc


_NC_CACHE: dict = {}


def _get_nc() -> bass.Bass:
    if "nc" not in _NC_CACHE:
        _NC_CACHE["nc"] = build_nc()
    return _NC_CACHE["nc"]


def make_in_maps(inputs: dict) -> list:
    x = np.asarray(inputs["input_features"], np.float32)
    pre_w = np.asarray(inputs["pre_w"], np.float32)
    pre_b = np.asarray(inputs["pre_b"], np.float32)
    q_params = np.asarray(inputs["q_params"], np.float32)
    post_w = np.asarray(inputs["post_w"], np.float32)
    post_b = np.asarray(inputs["post_b"], np.float32)

    M = _build_M(q_params)
    P = _build_P(post_w)
    mbd = np.zeros((128, 128), np.float32)
    pbd = np.zeros((128, 80), np.float32)
    for t in range(8):
        mbd[16 * t:16 * (t + 1), 16 * t:16 * (t + 1)] = M.T
        pbd[16 * t:16 * (t + 1), 10 * t:10 * (t + 1)] = P
    # pre_wt_sb[p, 4k+f] = pre_w[f, 128k+p]
    pre_wt = np.ascontiguousarray(
        pre_w.T.reshape(4, 128, 4).transpose(1, 0, 2).reshape(128, 16)
    ).astype(ml_dtypes.bfloat16)
    pre_b41 = np.ascontiguousarray(pre_b.reshape(4, 1), dtype=np.float32)
    post_b80 = np.ascontiguousarray(np.tile(post_b, 8).reshape(80, 1))
    trigb = np.ascontiguousarray(np.broadcast_to(
        np.array([3.0 * PI4, PI4], np.float32), (128, 2)))
    ident = np.eye(128, dtype=np.float32)

    xb = x.astype(ml_dtypes.bfloat16)
    consts = dict(pre_wt=pre_wt, pre_b41=pre_b41, mbd=mbd, pbd=pbd,
                  post_b80=post_b80, trigb=trigb, ident=ident)
    return [dict(x=xb[B * i:B * (i + 1)], **consts) for i in range(N_CORES)]


def unpack_out(dev_out: np.ndarray) -> np.ndarray:
    """[TILES, C, 128] device layout -> [B, C]."""
    return dev_out.transpose(0, 2, 1).reshape(B, C)


def run_on_device(inputs: dict, **kwargs):
    """Returns (full_output, BassKernelResults)."""
    nc = _get_nc()
    in_maps = make_in_maps(inputs)
    res = run_bass_kernel_spmd(nc, in_maps, core_ids=list(range(N_CORES)),
                               **kwargs)
    full = np.concatenate(
        [unpack_out(res.results[i]["out"]) for i in range(N_CORES)], 0)
    return np.ascontiguousarray(full, dtype=np.float32), res


def kernel(**inputs) -> np.ndarray:
    out, _ = run_on_device(inputs)
    return out



# revision 6
# speedup vs baseline: 1.1206x; 1.1206x over previous
"""Trainium2 Bass kernel for nn_DressedQuantumNet.

Math reformulation (exact, up to float rounding):
  pre_out = x @ pre_w.T + pre_b                  # [B,4]
  theta_w = (pi/4)*tanh(pre_out_w) + pi/4        # in (0, pi/2)
  v_w     = [cos theta_w, sin theta_w]           # per-qubit state (positive)
  psi     = v_0 (x) v_1 (x) v_2 (x) v_3          # [B,16] product state
  phi     = M @ psi        # M = fixed 16x16 matrix of the CNOT/RY circuit
  out     = (phi*phi)^T P + post_b  # P[i,c] = sum_w post_w[c,w] * z_w(i)

Device strategy (pure data parallel over 8 cores, 8192 samples each):
  - x bf16, loaded transposed via the DMA xbar on the sync queue ONLY
    (16 transposes of 512 samples; nothing else rides that queue, so the
    x load streams at the xbar rate ~292GB/s — it is the critical path).
  - pre-matmul is PE col-tiled: the 4 groups of a 2048-sample "quad" go
    to col-strips 0/32/64/96 of the PE array (tile_position), so their
    512-col matmuls run concurrently and the psum output is a dense
    [128, 512] tile (group j on partitions 32j..32j+3).
  - tanh is one [128,512] activation per quad (128-lane efficient).
  - the [feature, sample] -> [sample, feature] flip is 4 PE transposes
    per quad (bf16, into psum); NO SBUF->SBUF xbar transposes and no
    gpsimd memset (garbage rows/cols are simply never consumed).
  - trig on ScalarE reads the transposed psum directly (2x Sin with
    scale/bias folding cos); psi built with 3 broadcast-AP vector mults.
  - quantum circuit: PE transpose of psi -> [(tile,comp), sample], then
    block-diagonal M (16x16 x8) and P (16x10 x8) matmuls in fp32.
"""

import os
import sys

for _p in ("/opt/trn_rl_repo",):
    if os.path.isdir(_p) and _p not in sys.path:
        sys.path.insert(0, _p)

import math
import numpy as np
import ml_dtypes
from contextlib import ExitStack

import concourse.bass as bass
import concourse.bacc as bacc
import concourse.mybir as mybir
from concourse.tile import TileContext, add_dep_helper
from concourse.bass_utils import run_bass_kernel_spmd

F32 = mybir.dt.float32
BF16 = mybir.dt.bfloat16
AF = mybir.ActivationFunctionType
PI4 = math.pi / 4.0

N_CORES = 8
B_FULL, D, C = 65536, 512, 10
B = B_FULL // N_CORES          # 8192 samples per core
N_QUBITS, Q_DEPTH = 4, 6
GROUPS = 16                    # groups of 512 samples
QUADS = 4                      # quads of 4 groups (2048 samples)


# ---------------------------------------------------------------- host math
def _apply_1q(state, gate, wire):
    state = np.moveaxis(state, wire, 0)
    state = np.tensordot(gate, state, axes=((1,), (0,)))
    return np.moveaxis(state, 0, wire)


def _apply_cnot(state, ctrl, tgt):
    state = np.moveaxis(state, (ctrl, tgt), (0, 1))
    state = np.stack([state[0], state[1][::-1]], axis=0)
    return np.moveaxis(state, (0, 1), (ctrl, tgt))


def _ry(theta):
    c, s = np.cos(theta * 0.5), np.sin(theta * 0.5)
    return np.array([[c, -s], [s, c]])


def _build_M(q_params: np.ndarray) -> np.ndarray:
    """16x16 matrix of the fixed part of the circuit (after the per-sample
    RY layer): 6 repetitions of [CNOT(0,1), CNOT(2,3), CNOT(1,2), RY layer]."""
    qw = np.asarray(q_params, np.float64).reshape(Q_DEPTH, N_QUBITS)
    M = np.zeros((16, 16), np.float64)
    for i in range(16):
        state = np.zeros(16, np.float64)
        state[i] = 1.0
        state = state.reshape((2,) * N_QUBITS)
        for k in range(Q_DEPTH):
            for a in range(0, N_QUBITS - 1, 2):
                state = _apply_cnot(state, a, a + 1)
            for a in range(1, N_QUBITS - 1, 2):
                state = _apply_cnot(state, a, a + 1)
            for w in range(N_QUBITS):
                state = _apply_1q(state, _ry(qw[k, w]), w)
        M[:, i] = state.reshape(16)
    return M


def _build_P(post_w: np.ndarray) -> np.ndarray:
    """P[i, c] = sum_w post_w[c, w] * z_w(i), where z_w(i) flips sign with
    bit (3-w) of the state index i (axis 0 of the state = qubit 0)."""
    post_w = np.asarray(post_w, np.float64)
    i = np.arange(16)
    z = np.stack([1.0 - 2.0 * ((i >> (3 - w)) & 1) for w in range(N_QUBITS)], 1)
    return z @ post_w.T  # [16, 10]


# ---------------------------------------------------------------- bass build
def build_nc(sim_compat: bool = False) -> bass.Bass:
    # Bacc (not raw Bass): its finalize() runs generate_event_semaphores,
    # which splits multi-semaphore waits to satisfy the TRN2 one-wait-per-
    # instruction ISA limit.
    nc = bacc.Bacc(None)
    x = nc.dram_tensor("x", [B, D], BF16, kind="ExternalInput")
    # pre_wt[p, 32k + f] = pre_w[f, 128k+p] (f<4; cols 4..31 of each chunk 0)
    pre_wt = nc.dram_tensor("pre_wt", [128, 128], BF16, kind="ExternalInput")
    pre_b128 = nc.dram_tensor("pre_b128", [128, 1], F32, kind="ExternalInput")
    mbd = nc.dram_tensor("mbd", [128, 128], F32, kind="ExternalInput")
    pbd = nc.dram_tensor("pbd", [128, 80], F32, kind="ExternalInput")
    post_b80 = nc.dram_tensor("post_b80", [80, 1], F32, kind="ExternalInput")
    trigb = nc.dram_tensor("trigb", [128, 2], F32, kind="ExternalInput")
    identb = nc.dram_tensor("identb", [128, 128], BF16, kind="ExternalInput")
    identf = nc.dram_tensor("identf", [128, 128], F32, kind="ExternalInput")
    # transposed on device: out[(tile,comp) partition, quad*256+slab*128+p]
    out = nc.dram_tensor("out", [80, 1024], F32, kind="ExternalOutput")

    with ExitStack() as ctx:
        tc = ctx.enter_context(TileContext(nc))
        consts = ctx.enter_context(tc.tile_pool(name="consts", bufs=1))
        # all 16 xt group tiles stay resident (8 MB) — no WAR waits on the
        # transpose DMAs
        xt_pool = ctx.enter_context(tc.tile_pool(name="xt", bufs=GROUPS))
        work = ctx.enter_context(tc.tile_pool(name="work", bufs=2))
        ps_po = ctx.enter_context(tc.tile_pool(name="ps_po", space="PSUM", bufs=2))
        ps_th = ctx.enter_context(tc.tile_pool(name="ps_th", space="PSUM", bufs=2))
        ps_ct = ctx.enter_context(tc.tile_pool(name="ps_ct", space="PSUM", bufs=1))

        pre_wt_sb = consts.tile([128, 128], BF16)
        nc.scalar.dma_start(pre_wt_sb, pre_wt[:, :])
        pre_b_sb = consts.tile([128, 1], F32)
        nc.scalar.dma_start(pre_b_sb, pre_b128[:, :])
        mbd_sb = consts.tile([128, 128], F32)
        nc.scalar.dma_start(mbd_sb, mbd[:, :])
        pbd_sb = consts.tile([128, 80], F32)
        nc.scalar.dma_start(pbd_sb, pbd[:, :])
        pb80_sb = consts.tile([80, 1], F32)
        nc.scalar.dma_start(pb80_sb, post_b80[:, :])
        trigb_sb = consts.tile([128, 2], F32)
        nc.scalar.dma_start(trigb_sb, trigb[:, :])
        idb_sb = consts.tile([128, 128], BF16)
        nc.scalar.dma_start(idb_sb, identb[:, :])
        idf_sb = consts.tile([128, 128], F32)
        last_const = nc.scalar.dma_start(idf_sb, identf[:, :])

        out2_sb = consts.tile([80, 1024], F32)
        # tanh staging for all samples, bf16: [128 rows (32j+f), 512 cols/quad]
        tanh_sb = consts.tile([128, 4 * 512], BF16)

        # pin the activation table to silu_and_others once: it contains
        # silu+tanh+sin+square+identity, so no further table loads happen.
        # (CoreSim can't evaluate Silu; the sim build substitutes Tanh —
        # the value is unused either way.)
        silu_sb = consts.tile([128, 1], F32)
        nc.scalar.activation(silu_sb, trigb_sb[:, 0:1],
                             AF.Tanh if sim_compat else AF.Silu)

        # ---- all 16 x transposes up-front on the sync queue (the critical
        # path); group g covers samples 512g..512(g+1)
        xts = []
        for g in range(GROUPS):
            xt = xt_pool.tile([128, 4 * 512], BF16, name="xt", tag="xt")
            # one xbar transpose per 512-sample group with a fully
            # contiguous 512KB DRAM source: out[p, k, b] = x[b, 128k+p]
            # NOTE: all xbar transposes must stay on ONE HWDGE queue —
            # concurrent transpose streams on the SP and ACT rings
            # corrupt data through the shared xbar (measured twice).
            xpose = nc.sync.dma_start(
                xt[:, :].rearrange("p (k b) -> p k b", k=4),
                x[512 * g:512 * (g + 1), :],
                transpose=True)
            # keep all plain copies scheduled before all xbar transposes
            # (every copy<->transpose transition serializes the DMA ring)
            add_dep_helper(last_const.ins, xpose.ins, sync=False,
                           reason="consts before xbar transposes")
            xts.append(xt)

        for q in range(QUADS):
            # ---- pre-net: 4 groups col-tiled onto PE strips 0/32/64/96.
            # po[32j + f, s] = pre_out feature f of sample 512*(4q+j) + s.
            # psum pending-zero state is per-partition, so each col-strip
            # opens/closes its own accumulation group (start on its k=0,
            # stop on its k=3); skip_group_check silences the bank-granular
            # build-time checker which doesn't model per-strip groups.
            po = ps_po.tile([128, 512], F32, name="po", tag="po")
            for k in range(4):
                for j in range(4):
                    nc.tensor.matmul(
                        po[32 * j:32 * (j + 1), :],
                        lhsT=pre_wt_sb[:, 32 * k:32 * (k + 1)],
                        rhs=xts[4 * q + j][:, :].rearrange(
                            "p (k b) -> p k b", k=4)[:, k, :],
                        start=(k == 0), stop=(k == 3),
                        tile_position=(0, 32 * j),
                        skip_group_check=True)
            # fused bias + tanh on the whole quad, bf16 out
            tq = tanh_sb[:, 512 * q:512 * (q + 1)]
            nc.scalar.activation(tq, po, AF.Tanh, bias=pre_b_sb[:, :])

            # ---- flip to sample-major: 4 PE transposes [128,128] -> psum.
            # thT[p, 128k + 32j + f] = tanh feature f of sample
            # 512*(4q+j) + 128k + p  (cols 32j+4..32j+31 are garbage)
            thT = ps_th.tile([128, 512], BF16, name="thT", tag="thT")
            for k in range(4):
                nc.tensor.transpose(
                    thT[:, 128 * k:128 * (k + 1)],
                    tq[:, 128 * k:128 * (k + 1)], idb_sb[:, :])

            # ---- trig: cos/sin of theta = PI4*t + {3pi/4, pi/4}
            # cs[p, ((k,j), w, x)]; (k,j) flattened to tile index kj.
            cs = work.tile([128, 128], F32, name="cs", tag="cs")
            cs5 = cs[:, :].rearrange("p (k j w x) -> p k j w x",
                                     k=4, j=4, w=4, x=2)
            thT4 = thT[:, :].rearrange("p (k j w) -> p k j w", k=4, j=4)
            nc.scalar.activation(cs5[:, :, :, :, 0], thT4[:, :, :, 0:4],
                                 AF.Sin, bias=trigb_sb[:, 0:1], scale=PI4)
            nc.scalar.activation(cs5[:, :, :, :, 1], thT4[:, :, :, 0:4],
                                 AF.Sin, bias=trigb_sb[:, 1:2], scale=PI4)

            # ---- psi = v0 (x) v1 (x) v2 (x) v3 per tile kj
            cs4 = cs[:, :].rearrange("p (kj w x) -> p kj w x", w=4, x=2)
            v01 = work.tile([128, 64], F32, name="v01", tag="v01")
            v23 = work.tile([128, 64], F32, name="v23", tag="v23")
            nc.vector.tensor_tensor(
                out=v01[:, :].rearrange("p (t a b) -> p t a b", a=2, b=2),
                in0=cs4[:, :, 0, :].unsqueeze(3).broadcast_to((128, 16, 2, 2)),
                in1=cs4[:, :, 1, :].unsqueeze(2).broadcast_to((128, 16, 2, 2)),
                op=mybir.AluOpType.mult)
            nc.vector.tensor_tensor(
                out=v23[:, :].rearrange("p (t a b) -> p t a b", a=2, b=2),
                in0=cs4[:, :, 2, :].unsqueeze(3).broadcast_to((128, 16, 2, 2)),
                in1=cs4[:, :, 3, :].unsqueeze(2).broadcast_to((128, 16, 2, 2)),
                op=mybir.AluOpType.mult)
            psi = work.tile([128, 256], F32, name="psi", tag="psi")
            nc.vector.tensor_tensor(
                out=psi[:, :].rearrange("p (t a b) -> p t a b", a=4, b=4),
                in0=v01[:, :].rearrange("p (t i) -> p t i", i=4)
                    .unsqueeze(3).broadcast_to((128, 16, 4, 4)),
                in1=v23[:, :].rearrange("p (t i) -> p t i", i=4)
                    .unsqueeze(2).broadcast_to((128, 16, 4, 4)),
                op=mybir.AluOpType.mult)

            # ---- quantum circuit, 2 slabs of 8 tiles -> one 256-col matmul
            psiT = work.tile([128, 256], F32, name="psiT", tag="psiT")
            for h in range(2):
                psiT_ps = ps_ct.tile([128, 128], F32, name="psiT_ps", tag="pT")
                nc.tensor.transpose(
                    psiT_ps, psi[:, 128 * h:128 * (h + 1)], idf_sb[:, :])
                nc.vector.tensor_copy(psiT[:, 128 * h:128 * (h + 1)], psiT_ps)
            phiT_ps = ps_ct.tile([128, 256], F32, name="phiT_ps", tag="phT")
            nc.tensor.matmul(phiT_ps, lhsT=mbd_sb[:, :], rhs=psiT[:, :],
                             start=True, stop=True)
            phi2 = work.tile([128, 256], F32, name="phi2", tag="phi2")
            nc.scalar.activation(phi2, phiT_ps, AF.Square)
            o10_ps = ps_ct.tile([80, 256], F32, name="o10_ps", tag="o10")
            nc.tensor.matmul(o10_ps, lhsT=pbd_sb[:, :], rhs=phi2,
                             start=True, stop=True)
            # bias-add lands directly in the transposed output staging tile
            nc.scalar.activation(out2_sb[:, 256 * q:256 * (q + 1)],
                                 o10_ps, AF.Identity, bias=pb80_sb[:, :])

        # single store at the very end (keeps the transpose stream free of
        # plain-copy ring transitions); the host unpacks the layout
        nc.scalar.dma_start(out[:, :], out2_sb[:, :])

    nc.finalize()  # bacc: register alloc + event-semaphore wait splitting
    return nc


_NC_CACHE: dict = {}


def _get_nc() -> bass.Bass:
    if "nc" not in _NC_CACHE:
        _NC_CACHE["nc"] = build_nc()
    return _NC_CACHE["nc"]


def make_in_maps(inputs: dict) -> list:
    x = np.asarray(inputs["input_features"], np.float32)
    pre_w = np.asarray(inputs["pre_w"], np.float32)
    pre_b = np.asarray(inputs["pre_b"], np.float32)
    q_params = np.asarray(inputs["q_params"], np.float32)
    post_w = np.asarray(inputs["post_w"], np.float32)
    post_b = np.asarray(inputs["post_b"], np.float32)

    M = _build_M(q_params)
    P = _build_P(post_w)
    mbd = np.zeros((128, 128), np.float32)
    pbd = np.zeros((128, 80), np.float32)
    for t in range(8):
        mbd[16 * t:16 * (t + 1), 16 * t:16 * (t + 1)] = M.T
        pbd[16 * t:16 * (t + 1), 10 * t:10 * (t + 1)] = P
    # pre_wt[p, 32k + f] = pre_w[f, 128k+p], zero-padded to 32 cols/chunk
    pre_wt = np.zeros((128, 128), np.float32)
    for k in range(4):
        pre_wt[:, 32 * k:32 * k + 4] = pre_w[:, 128 * k:128 * (k + 1)].T
    pre_wt = pre_wt.astype(ml_dtypes.bfloat16)
    # pre_b on rows 32j+f (any j): broadcast per strip
    pre_b128 = np.zeros((128, 1), np.float32)
    for j in range(4):
        pre_b128[32 * j:32 * j + 4, 0] = pre_b
    post_b80 = np.ascontiguousarray(np.tile(post_b, 8).reshape(80, 1))
    trigb = np.ascontiguousarray(np.broadcast_to(
        np.array([3.0 * PI4, PI4], np.float32), (128, 2)))
    identb = np.eye(128, dtype=ml_dtypes.bfloat16)
    identf = np.eye(128, dtype=np.float32)

    xb = x.astype(ml_dtypes.bfloat16)
    consts = dict(pre_wt=pre_wt, pre_b128=pre_b128, mbd=mbd, pbd=pbd,
                  post_b80=post_b80, trigb=trigb, identb=identb,
                  identf=identf)
    return [dict(x=xb[B * i:B * (i + 1)], **consts) for i in range(N_CORES)]


def unpack_out(dev_out: np.ndarray) -> np.ndarray:
    """[80, 1024] device layout -> [B, C].

    dev[(m,c), (q,h,p)] = out of sample 512*(4q+j) + 128k + p, class c,
    where kj = 8h + m, k = kj//4, j = kj%4.
    """
    d = dev_out.reshape(8, C, QUADS, 2, 128)          # m, c, q, h, p
    d = d.transpose(2, 3, 0, 4, 1)                    # q, h, m, p, c
    d = d.reshape(QUADS, 16, 128, C)                  # q, kj, p, c
    d = d.reshape(QUADS, 4, 4, 128, C)                # q, k, j, p, c
    d = d.transpose(0, 2, 1, 3, 4)                    # q, j, k, p, c
    return np.ascontiguousarray(d.reshape(B, C))


def run_on_device(inputs: dict, **kwargs):
    """Returns (full_output, BassKernelResults)."""
    nc = _get_nc()
    in_maps = make_in_maps(inputs)
    res = run_bass_kernel_spmd(nc, in_maps, core_ids=list(range(N_CORES)),
                               **kwargs)
    full = np.concatenate(
        [unpack_out(res.results[i]["out"]) for i in range(N_CORES)], 0)
    return np.ascontiguousarray(full, dtype=np.float32), res


def kernel(**inputs) -> np.ndarray:
    out, _ = run_on_device(inputs)
    return out


# revision 16
# speedup vs baseline: 1.2735x; 1.1364x over previous
"""Trainium2 Bass kernel for nn_DressedQuantumNet.

Math reformulation (exact, up to float rounding):
  pre_out = x @ pre_w.T + pre_b                  # [B,4]
  theta_w = (pi/4)*tanh(pre_out_w) + pi/4        # in (0, pi/2)
  v_w     = [cos theta_w, sin theta_w]           # per-qubit state (positive)
  psi     = v_0 (x) v_1 (x) v_2 (x) v_3          # [B,16] product state
  phi     = M @ psi        # M = fixed 16x16 matrix of the CNOT/RY circuit
  out     = (phi*phi)^T P + post_b  # P[i,c] = sum_w post_w[c,w] * z_w(i)

Device strategy (pure data parallel over 8 cores, 8192 samples each):
  - x bf16, loaded transposed via the DMA xbar on the sync queue ONLY
    (16 transposes of 512 samples; nothing else rides that queue, so the
    x load streams at the xbar rate ~292GB/s — it is the critical path).
  - pre-matmul is PE col-tiled: the 4 groups of a 2048-sample "quad" go
    to col-strips 0/32/64/96 of the PE array (tile_position), so their
    512-col matmuls run concurrently and the psum output is a dense
    [128, 512] tile (group j on partitions 32j..32j+3).
  - tanh is one [128,512] activation per quad (128-lane efficient).
  - the [feature, sample] -> [sample, feature] flip is 4 PE transposes
    per quad (bf16, into psum); NO SBUF->SBUF xbar transposes and no
    gpsimd memset (garbage rows/cols are simply never consumed).
  - trig on ScalarE reads the transposed psum directly (2x Sin with
    scale/bias folding cos); psi built with 3 broadcast-AP vector mults.
  - quantum circuit: PE transpose of psi -> [(tile,comp), sample], then
    block-diagonal M (16x16 x8) and P (16x10 x8) matmuls in fp32.
"""

import os
import sys

for _p in ("/opt/trn_rl_repo",):
    if os.path.isdir(_p) and _p not in sys.path:
        sys.path.insert(0, _p)

import math
import numpy as np
import ml_dtypes
from contextlib import ExitStack

import concourse.bass as bass
import concourse.bacc as bacc
import concourse.mybir as mybir
from concourse.tile import TileContext, add_dep_helper
from concourse.bass_utils import run_bass_kernel_spmd

F32 = mybir.dt.float32
F32R = mybir.dt.float32r
BF16 = mybir.dt.bfloat16
AF = mybir.ActivationFunctionType
PI4 = math.pi / 4.0

PIN_QUEUES = frozenset(("tensor", "sync", "scalar", "vector"))
N_CORES = 8
B_FULL, D, C = 65536, 512, 10
B = B_FULL // N_CORES          # 8192 samples per core
N_QUBITS, Q_DEPTH = 4, 6
GROUPS = 16                    # groups of 512 samples
QUADS = 4                      # quads of 4 groups (2048 samples)


# ---------------------------------------------------------------- host math
def _apply_1q(state, gate, wire):
    state = np.moveaxis(state, wire, 0)
    state = np.tensordot(gate, state, axes=((1,), (0,)))
    return np.moveaxis(state, 0, wire)


def _apply_cnot(state, ctrl, tgt):
    state = np.moveaxis(state, (ctrl, tgt), (0, 1))
    state = np.stack([state[0], state[1][::-1]], axis=0)
    return np.moveaxis(state, (0, 1), (ctrl, tgt))


def _ry(theta):
    c, s = np.cos(theta * 0.5), np.sin(theta * 0.5)
    return np.array([[c, -s], [s, c]])


def _build_M(q_params: np.ndarray) -> np.ndarray:
    """16x16 matrix of the fixed part of the circuit (after the per-sample
    RY layer): 6 repetitions of [CNOT(0,1), CNOT(2,3), CNOT(1,2), RY layer]."""
    qw = np.asarray(q_params, np.float64).reshape(Q_DEPTH, N_QUBITS)
    M = np.zeros((16, 16), np.float64)
    for i in range(16):
        state = np.zeros(16, np.float64)
        state[i] = 1.0
        state = state.reshape((2,) * N_QUBITS)
        for k in range(Q_DEPTH):
            for a in range(0, N_QUBITS - 1, 2):
                state = _apply_cnot(state, a, a + 1)
            for a in range(1, N_QUBITS - 1, 2):
                state = _apply_cnot(state, a, a + 1)
            for w in range(N_QUBITS):
                state = _apply_1q(state, _ry(qw[k, w]), w)
        M[:, i] = state.reshape(16)
    return M


def _build_P(post_w: np.ndarray) -> np.ndarray:
    """P[i, c] = sum_w post_w[c, w] * z_w(i), where z_w(i) flips sign with
    bit (3-w) of the state index i (axis 0 of the state = qubit 0)."""
    post_w = np.asarray(post_w, np.float64)
    i = np.arange(16)
    z = np.stack([1.0 - 2.0 * ((i >> (3 - w)) & 1) for w in range(N_QUBITS)], 1)
    return z @ post_w.T  # [16, 10]


# ---------------------------------------------------------------- bass build
def build_nc(sim_compat: bool = False) -> bass.Bass:
    # Bacc (not raw Bass): its finalize() runs generate_event_semaphores,
    # which splits multi-semaphore waits to satisfy the TRN2 one-wait-per-
    # instruction ISA limit.
    nc = bacc.Bacc(None)
    x = nc.dram_tensor("x", [B, D], BF16, kind="ExternalInput")
    # pre_wt[p, 32k + f] = pre_w[f, 128k+p] (f<4; cols 4..31 of each chunk 0)
    pre_wt = nc.dram_tensor("pre_wt", [128, 128], BF16, kind="ExternalInput")
    pre_b128 = nc.dram_tensor("pre_b128", [128, 1], F32, kind="ExternalInput")
    mbd = nc.dram_tensor("mbd", [128, 128], F32R, kind="ExternalInput")
    pbd = nc.dram_tensor("pbd", [128, 80], F32R, kind="ExternalInput")
    post_b80 = nc.dram_tensor("post_b80", [80, 1], F32, kind="ExternalInput")
    trigb = nc.dram_tensor("trigb", [128, 2], F32, kind="ExternalInput")
    identb = nc.dram_tensor("identb", [128, 128], BF16, kind="ExternalInput")
    identf = nc.dram_tensor("identf", [128, 128], F32, kind="ExternalInput")
    # transposed on device: out[(tile,comp) partition, quad*256+slab*128+p]
    out = nc.dram_tensor("out", [80, 1024], F32, kind="ExternalOutput")

    with ExitStack() as ctx:
        tc = ctx.enter_context(TileContext(nc))
        consts = ctx.enter_context(tc.tile_pool(name="consts", bufs=1))
        # all 16 xt group tiles stay resident (8 MB) — no WAR waits on the
        # transpose DMAs
        xt_pool = ctx.enter_context(tc.tile_pool(name="xt", bufs=GROUPS))
        work = ctx.enter_context(tc.tile_pool(name="work", bufs=2))
        ps_po = ctx.enter_context(tc.tile_pool(name="ps_po", space="PSUM", bufs=2))
        ps_th = ctx.enter_context(tc.tile_pool(name="ps_th", space="PSUM", bufs=2))
        ps_ct = ctx.enter_context(tc.tile_pool(name="ps_ct", space="PSUM", bufs=1))

        last_on = {}

        def pin(engine_key, bass_ins):
            if engine_key not in PIN_QUEUES:
                return bass_ins
            prev = last_on.get(engine_key)
            if prev is not None:
                # add_dep_helper(a, b) = a depends on b: order bass_ins AFTER prev
                add_dep_helper(bass_ins.ins, prev.ins, sync=False,
                               reason="queue order pin")
            last_on[engine_key] = bass_ins
            return bass_ins

        pre_wt_sb = consts.tile([128, 128], BF16)
        pin("scalar", nc.scalar.dma_start(pre_wt_sb, pre_wt[:, :]))
        pre_b_sb = consts.tile([128, 1], F32)
        pin("scalar", nc.scalar.dma_start(pre_b_sb, pre_b128[:, :]))
        mbd_sb = consts.tile([128, 128], F32R)
        pin("scalar", nc.scalar.dma_start(mbd_sb, mbd[:, :]))
        pbd_sb = consts.tile([128, 80], F32R)
        pin("scalar", nc.scalar.dma_start(pbd_sb, pbd[:, :]))
        pb80_sb = consts.tile([80, 1], F32)
        pin("scalar", nc.scalar.dma_start(pb80_sb, post_b80[:, :]))
        trigb_sb = consts.tile([128, 2], F32)
        pin("scalar", nc.scalar.dma_start(trigb_sb, trigb[:, :]))
        idb_sb = consts.tile([128, 128], BF16)
        pin("scalar", nc.scalar.dma_start(idb_sb, identb[:, :]))
        idf_sb = consts.tile([128, 128], F32)
        last_const = pin("scalar", nc.scalar.dma_start(idf_sb, identf[:, :]))

        out2_sb = consts.tile([80, 1024], F32)
        # tanh staging for all samples, bf16: [128 rows (32j+f), 512 cols/quad]
        tanh_sb = consts.tile([128, 4 * 512], BF16)

        # pin the activation table to silu_and_others once: it contains
        # silu+tanh+sin+square+identity, so no further table loads happen.
        # (CoreSim can't evaluate Silu; the sim build substitutes Tanh —
        # the value is unused either way.)
        silu_sb = consts.tile([128, 1], F32)
        pin("scalar", nc.scalar.activation(silu_sb, trigb_sb[:, 0:1],
                                           AF.Tanh if sim_compat else AF.Silu))

        # NOTE on pin(): the runtime executes each engine queue strictly
        # in-order, and the Tile scheduler's simulated timeline mis-models
        # the long xbar transposes (it hoisted all pre-matmuls ahead of all
        # circuit matmuls, pushing every quad's phase 2 behind the whole
        # 60us DMA stream).  pin() forces each engine queue into emission
        # order with free order-only (sync=False) dep edges.

        # ---- all 16 x transposes up-front on the sync queue (the critical
        # path); group g covers samples 512g..512(g+1)
        xts = []
        for g in range(GROUPS):
            xt = xt_pool.tile([128, 4 * 512], BF16, name="xt", tag="xt")
            # one xbar transpose per 512-sample group with a fully
            # contiguous 512KB DRAM source: out[p, k, b] = x[b, 128k+p]
            # NOTE: all xbar transposes must stay on ONE HWDGE queue —
            # concurrent transpose streams on the SP and ACT rings
            # corrupt data through the shared xbar (measured twice).
            xpose = nc.sync.dma_start(
                xt[:, :].rearrange("p (k b) -> p k b", k=4),
                x[512 * g:512 * (g + 1), :],
                transpose=True)
            # keep all plain copies scheduled before all xbar transposes
            # (every copy<->transpose transition serializes the DMA ring)
            add_dep_helper(xpose.ins, last_const.ins, sync=False,
                           reason="consts before xbar transposes")
            pin("sync", xpose)
            xts.append(xt)

        for q in range(QUADS):
            # ---- pre-net: 4 groups col-tiled onto PE strips 0/32/64/96.
            # po[32j + f, s] = pre_out feature f of sample 512*(4q+j) + s.
            # psum pending-zero state is per-partition, so each col-strip
            # opens/closes its own accumulation group (start on its k=0,
            # stop on its k=3); skip_group_check silences the bank-granular
            # build-time checker which doesn't model per-strip groups.
            # Strip-major (j outer) order: strip j's 4-matmul chain starts
            # as soon as ITS group's transpose lands, and neighboring
            # strips' chains overlap on the PE (distinct col-groups).
            po = ps_po.tile([128, 512], F32, name="po", tag="po")
            for j in range(4):
                for k in range(4):
                    mm = nc.tensor.matmul(
                        po[32 * j:32 * (j + 1), :],
                        lhsT=pre_wt_sb[:, 32 * k:32 * (k + 1)],
                        rhs=xts[4 * q + j][:, :].rearrange(
                            "p (k b) -> p k b", k=4)[:, k, :],
                        start=(k == 0), stop=(k == 3),
                        tile_position=(0, 32 * j),
                        skip_group_check=True)
                    pin("tensor", mm)
            # fused bias + tanh on the whole quad, bf16 out
            tq = tanh_sb[:, 512 * q:512 * (q + 1)]
            pin("scalar", nc.scalar.activation(tq, po, AF.Tanh,
                                               bias=pre_b_sb[:, :]))

            # ---- flip to sample-major: 4 PE transposes [128,128] -> psum.
            # thT[p, 128k + 32j + f] = tanh feature f of sample
            # 512*(4q+j) + 128k + p  (cols 32j+4..32j+31 are garbage)
            thT = ps_th.tile([128, 512], BF16, name="thT", tag="thT")
            for k in range(4):
                pin("tensor", nc.tensor.transpose(
                    thT[:, 128 * k:128 * (k + 1)],
                    tq[:, 128 * k:128 * (k + 1)], idb_sb[:, :]))

            # ---- trig: cos/sin of theta = PI4*t + {3pi/4, pi/4}
            # cs[p, ((k,j), w, x)]; (k,j) flattened to tile index kj.
            cs = work.tile([128, 128], F32, name="cs", tag="cs")
            cs5 = cs[:, :].rearrange("p (k j w x) -> p k j w x",
                                     k=4, j=4, w=4, x=2)
            thT4 = thT[:, :].rearrange("p (k j w) -> p k j w", k=4, j=4)
            pin("scalar", nc.scalar.activation(
                cs5[:, :, :, :, 0], thT4[:, :, :, 0:4],
                AF.Sin, bias=trigb_sb[:, 0:1], scale=PI4))
            pin("scalar", nc.scalar.activation(
                cs5[:, :, :, :, 1], thT4[:, :, :, 0:4],
                AF.Sin, bias=trigb_sb[:, 1:2], scale=PI4))

            # ---- psi = v0 (x) v1 (x) v2 (x) v3 per tile kj
            cs4 = cs[:, :].rearrange("p (kj w x) -> p kj w x", w=4, x=2)
            v01 = work.tile([128, 64], F32, name="v01", tag="v01")
            v23 = work.tile([128, 64], F32, name="v23", tag="v23")
            pin("vector", nc.vector.tensor_tensor(
                out=v01[:, :].rearrange("p (t a b) -> p t a b", a=2, b=2),
                in0=cs4[:, :, 0, :].unsqueeze(3).broadcast_to((128, 16, 2, 2)),
                in1=cs4[:, :, 1, :].unsqueeze(2).broadcast_to((128, 16, 2, 2)),
                op=mybir.AluOpType.mult))
            pin("vector", nc.vector.tensor_tensor(
                out=v23[:, :].rearrange("p (t a b) -> p t a b", a=2, b=2),
                in0=cs4[:, :, 2, :].unsqueeze(3).broadcast_to((128, 16, 2, 2)),
                in1=cs4[:, :, 3, :].unsqueeze(2).broadcast_to((128, 16, 2, 2)),
                op=mybir.AluOpType.mult))
            psi = work.tile([128, 256], F32, name="psi", tag="psi")
            pin("vector", nc.vector.tensor_tensor(
                out=psi[:, :].rearrange("p (t a b) -> p t a b", a=4, b=4),
                in0=v01[:, :].rearrange("p (t i) -> p t i", i=4)
                    .unsqueeze(3).broadcast_to((128, 16, 4, 4)),
                in1=v23[:, :].rearrange("p (t i) -> p t i", i=4)
                    .unsqueeze(2).broadcast_to((128, 16, 4, 4)),
                op=mybir.AluOpType.mult))

            # ---- quantum circuit, 2 slabs of 8 tiles -> one 256-col matmul
            # (M and P run as float32r: full-rate rows at N>=256, vs fp32's
            # half-rate double LOW/HIGH pass)
            psiT = work.tile([128, 256], F32R, name="psiT", tag="psiT")
            for h in range(2):
                psiT_ps = ps_ct.tile([128, 128], F32, name="psiT_ps", tag="pT")
                pin("tensor", nc.tensor.transpose(
                    psiT_ps, psi[:, 128 * h:128 * (h + 1)], idf_sb[:, :]))
                pin("vector", nc.vector.tensor_copy(
                    psiT[:, 128 * h:128 * (h + 1)], psiT_ps))
            phiT_ps = ps_ct.tile([128, 256], F32, name="phiT_ps", tag="phT")
            pin("tensor", nc.tensor.matmul(
                phiT_ps, lhsT=mbd_sb[:, :], rhs=psiT[:, :],
                start=True, stop=True))
            phi2 = work.tile([128, 256], F32R, name="phi2", tag="phi2")
            pin("scalar", nc.scalar.activation(phi2, phiT_ps, AF.Square))
            o10_ps = ps_ct.tile([80, 256], F32, name="o10_ps", tag="o10")
            pin("tensor", nc.tensor.matmul(
                o10_ps, lhsT=pbd_sb[:, :], rhs=phi2[:, :],
                start=True, stop=True))
            # bias-add lands directly in the transposed output staging tile
            pin("scalar", nc.scalar.activation(
                out2_sb[:, 256 * q:256 * (q + 1)],
                o10_ps, AF.Identity, bias=pb80_sb[:, :]))

        # single store at the very end (keeps the transpose stream free of
        # plain-copy ring transitions); the host unpacks the layout
        pin("scalar", nc.scalar.dma_start(out[:, :], out2_sb[:, :]))

    nc.finalize()  # bacc: register alloc + event-semaphore wait splitting
    return nc


_NC_CACHE: dict = {}


def _get_nc() -> bass.Bass:
    if "nc" not in _NC_CACHE:
        _NC_CACHE["nc"] = build_nc()
    return _NC_CACHE["nc"]


def make_in_maps(inputs: dict) -> list:
    x = np.asarray(inputs["input_features"], np.float32)
    pre_w = np.asarray(inputs["pre_w"], np.float32)
    pre_b = np.asarray(inputs["pre_b"], np.float32)
    q_params = np.asarray(inputs["q_params"], np.float32)
    post_w = np.asarray(inputs["post_w"], np.float32)
    post_b = np.asarray(inputs["post_b"], np.float32)

    M = _build_M(q_params)
    P = _build_P(post_w)
    mbd = np.zeros((128, 128), np.float32)
    pbd = np.zeros((128, 80), np.float32)
    for t in range(8):
        mbd[16 * t:16 * (t + 1), 16 * t:16 * (t + 1)] = M.T
        pbd[16 * t:16 * (t + 1), 10 * t:10 * (t + 1)] = P
    # pre_wt[p, 32k + f] = pre_w[f, 128k+p], zero-padded to 32 cols/chunk
    pre_wt = np.zeros((128, 128), np.float32)
    for k in range(4):
        pre_wt[:, 32 * k:32 * k + 4] = pre_w[:, 128 * k:128 * (k + 1)].T
    pre_wt = pre_wt.astype(ml_dtypes.bfloat16)
    # pre_b on rows 32j+f (any j): broadcast per strip
    pre_b128 = np.zeros((128, 1), np.float32)
    for j in range(4):
        pre_b128[32 * j:32 * j + 4, 0] = pre_b
    post_b80 = np.ascontiguousarray(np.tile(post_b, 8).reshape(80, 1))
    trigb = np.ascontiguousarray(np.broadcast_to(
        np.array([3.0 * PI4, PI4], np.float32), (128, 2)))
    identb = np.eye(128, dtype=ml_dtypes.bfloat16)
    identf = np.eye(128, dtype=np.float32)

    xb = x.astype(ml_dtypes.bfloat16)
    consts = dict(pre_wt=pre_wt, pre_b128=pre_b128, mbd=mbd, pbd=pbd,
                  post_b80=post_b80, trigb=trigb, identb=identb,
                  identf=identf)
    return [dict(x=xb[B * i:B * (i + 1)], **consts) for i in range(N_CORES)]


def unpack_out(dev_out: np.ndarray) -> np.ndarray:
    """[80, 1024] device layout -> [B, C].

    dev[(m,c), (q,h,p)] = out of sample 512*(4q+j) + 128k + p, class c,
    where kj = 8h + m, k = kj//4, j = kj%4.
    """
    d = dev_out.reshape(8, C, QUADS, 2, 128)          # m, c, q, h, p
    d = d.transpose(2, 3, 0, 4, 1)                    # q, h, m, p, c
    d = d.reshape(QUADS, 16, 128, C)                  # q, kj, p, c
    d = d.reshape(QUADS, 4, 4, 128, C)                # q, k, j, p, c
    d = d.transpose(0, 2, 1, 3, 4)                    # q, j, k, p, c
    return np.ascontiguousarray(d.reshape(B, C))


def run_on_device(inputs: dict, **kwargs):
    """Returns (full_output, BassKernelResults)."""
    nc = _get_nc()
    in_maps = make_in_maps(inputs)
    res = run_bass_kernel_spmd(nc, in_maps, core_ids=list(range(N_CORES)),
                               **kwargs)
    full = np.concatenate(
        [unpack_out(res.results[i]["out"]) for i in range(N_CORES)], 0)
    return np.ascontiguousarray(full, dtype=np.float32), res


def kernel(**inputs) -> np.ndarray:
    out, _ = run_on_device(inputs)
    return out


# revision 19
# speedup vs baseline: 1.3415x; 1.0534x over previous
"""Trainium2 Bass kernel for nn_DressedQuantumNet.

Math reformulation (exact, up to float rounding):
  pre_out = x @ pre_w.T + pre_b                  # [B,4]
  theta_w = (pi/4)*tanh(pre_out_w) + pi/4        # in (0, pi/2)
  v_w     = [cos theta_w, sin theta_w]           # per-qubit state (positive)
  psi     = v_0 (x) v_1 (x) v_2 (x) v_3          # [B,16] product state
  phi     = M @ psi        # M = fixed 16x16 matrix of the CNOT/RY circuit
  out     = (phi*phi)^T P + post_b  # P[i,c] = sum_w post_w[c,w] * z_w(i)

Device strategy (pure data parallel over 8 cores, 8192 samples each):
  - x bf16, loaded transposed via the DMA xbar on the sync queue; the 16
    group transposes are the critical path, so the sync queue carries
    only 3 packed const loads followed by the 16 transposes.
  - pre-matmul is PE col-tiled: the n groups of a unit go to col-strips
    32j of the PE array (tile_position), so their 512-col matmuls run
    concurrently and the psum output is a dense [32n, 512] tile.
  - units are tapered [4,4,4,2,2] groups so the last unit's dependent
    chain (the kernel tail after the final transpose) is short.
  - tanh is one [32n,512] activation per unit; the [feature, sample] ->
    [sample, feature] flip is 4 PE transposes per unit (bf16 -> psum);
    no SBUF->SBUF xbar transposes, no memset.
  - trig on ScalarE reads the transposed psum directly (2x Sin with
    scale/bias folding cos); psi built with 3 broadcast-AP vector mults.
  - quantum circuit: PE transpose of psi -> [(tile,comp), sample], then
    block-diagonal M (16x16 x8) and P (16x10 x8) matmuls in float32r
    (full-rate rows, ~fp32 accuracy).
  - per-unit output stores on the scalar queue overlap the x stream.
  - every engine queue is pinned to emission order (sync=False deps):
    queues execute in-order at runtime, and the Tile scheduler's DMA
    model otherwise reorders them badly around the long transposes.
"""

import os
import sys

for _p in ("/opt/trn_rl_repo",):
    if os.path.isdir(_p) and _p not in sys.path:
        sys.path.insert(0, _p)

import math
import numpy as np
import ml_dtypes
from contextlib import ExitStack

import concourse.bass as bass
import concourse.bacc as bacc
import concourse.mybir as mybir
from concourse.tile import TileContext, add_dep_helper
from concourse.bass_utils import run_bass_kernel_spmd

F32 = mybir.dt.float32
F32R = mybir.dt.float32r
BF16 = mybir.dt.bfloat16
AF = mybir.ActivationFunctionType
PI4 = math.pi / 4.0

PIN_QUEUES = frozenset(("tensor", "sync", "scalar", "vector"))
N_CORES = 8
B_FULL, D, C = 65536, 512, 10
B = B_FULL // N_CORES          # 8192 samples per core
N_QUBITS, Q_DEPTH = 4, 6
GROUPS = 16                    # groups of 512 samples
# units of n groups each (n col-strips of the PE); tapered so the last
# units' dependent chains are short
UNITS = [(0, 4), (4, 4), (8, 4), (12, 2), (14, 2)]

# f32 const blob column layout: pre_b | post_b | trigb | identity
FB_PREB = 0
FB_PB80 = 1
FB_TRIG = 2
FB_IDF = 4
FB_COLS = 4 + 128
# f32r const blob: mbd | pbd
RB_MBD = 0
RB_PBD = 128
RB_COLS = 128 + 80
# bf16 const blob: pre_wt | identity
BB_PWT = 0
BB_IDB = 128
BB_COLS = 256


# ---------------------------------------------------------------- host math
def _apply_1q(state, gate, wire):
    state = np.moveaxis(state, wire, 0)
    state = np.tensordot(gate, state, axes=((1,), (0,)))
    return np.moveaxis(state, 0, wire)


def _apply_cnot(state, ctrl, tgt):
    state = np.moveaxis(state, (ctrl, tgt), (0, 1))
    state = np.stack([state[0], state[1][::-1]], axis=0)
    return np.moveaxis(state, (0, 1), (ctrl, tgt))


def _ry(theta):
    c, s = np.cos(theta * 0.5), np.sin(theta * 0.5)
    return np.array([[c, -s], [s, c]])


def _build_M(q_params: np.ndarray) -> np.ndarray:
    """16x16 matrix of the fixed part of the circuit (after the per-sample
    RY layer): 6 repetitions of [CNOT(0,1), CNOT(2,3), CNOT(1,2), RY layer]."""
    qw = np.asarray(q_params, np.float64).reshape(Q_DEPTH, N_QUBITS)
    M = np.zeros((16, 16), np.float64)
    for i in range(16):
        state = np.zeros(16, np.float64)
        state[i] = 1.0
        state = state.reshape((2,) * N_QUBITS)
        for k in range(Q_DEPTH):
            for a in range(0, N_QUBITS - 1, 2):
                state = _apply_cnot(state, a, a + 1)
            for a in range(1, N_QUBITS - 1, 2):
                state = _apply_cnot(state, a, a + 1)
            for w in range(N_QUBITS):
                state = _apply_1q(state, _ry(qw[k, w]), w)
        M[:, i] = state.reshape(16)
    return M


def _build_P(post_w: np.ndarray) -> np.ndarray:
    """P[i, c] = sum_w post_w[c, w] * z_w(i), where z_w(i) flips sign with
    bit (3-w) of the state index i (axis 0 of the state = qubit 0)."""
    post_w = np.asarray(post_w, np.float64)
    i = np.arange(16)
    z = np.stack([1.0 - 2.0 * ((i >> (3 - w)) & 1) for w in range(N_QUBITS)], 1)
    return z @ post_w.T  # [16, 10]


# ---------------------------------------------------------------- bass build
def build_nc(sim_compat: bool = False) -> bass.Bass:
    # Bacc (not raw Bass): its finalize() runs generate_event_semaphores,
    # which splits multi-semaphore waits to satisfy the TRN2 one-wait-per-
    # instruction ISA limit.
    nc = bacc.Bacc(None)
    x = nc.dram_tensor("x", [B, D], BF16, kind="ExternalInput")
    fblob = nc.dram_tensor("fblob", [128, FB_COLS], F32, kind="ExternalInput")
    rblob = nc.dram_tensor("rblob", [128, RB_COLS], F32R, kind="ExternalInput")
    bblob = nc.dram_tensor("bblob", [128, BB_COLS], BF16, kind="ExternalInput")
    # transposed on device: out[(tile,comp) partition, 128*slab + p]
    out = nc.dram_tensor("out", [80, 1024], F32, kind="ExternalOutput")

    with ExitStack() as ctx:
        tc = ctx.enter_context(TileContext(nc))
        consts = ctx.enter_context(tc.tile_pool(name="consts", bufs=1))
        # all 16 xt group tiles stay resident (8 MB) — no WAR waits on the
        # transpose DMAs
        xt_pool = ctx.enter_context(tc.tile_pool(name="xt", bufs=GROUPS))
        work = ctx.enter_context(tc.tile_pool(name="work", bufs=2))
        ps_po = ctx.enter_context(tc.tile_pool(name="ps_po", space="PSUM", bufs=2))
        ps_th = ctx.enter_context(tc.tile_pool(name="ps_th", space="PSUM", bufs=2))
        ps_ct = ctx.enter_context(tc.tile_pool(name="ps_ct", space="PSUM", bufs=1))

        last_on = {}

        def pin(engine_key, bass_ins):
            if engine_key not in PIN_QUEUES:
                return bass_ins
            prev = last_on.get(engine_key)
            if prev is not None:
                # add_dep_helper(a, b) = a depends on b: bass_ins AFTER prev
                add_dep_helper(bass_ins.ins, prev.ins, sync=False,
                               reason="queue order pin")
            last_on[engine_key] = bass_ins
            return bass_ins

        # ---- consts: 3 packed loads on the SYNC queue, ahead of the
        # transposes (plain copies strictly before xbar transposes on the
        # same ring; nothing else ever rides this ring)
        fb_sb = consts.tile([128, FB_COLS], F32)
        pin("sync", nc.sync.dma_start(fb_sb, fblob[:, :]))
        rb_sb = consts.tile([128, RB_COLS], F32R)
        pin("sync", nc.sync.dma_start(rb_sb, rblob[:, :]))
        bb_sb = consts.tile([128, BB_COLS], BF16)
        pin("sync", nc.sync.dma_start(bb_sb, bblob[:, :]))
        pre_b_sb = fb_sb[:, FB_PREB:FB_PREB + 1]
        pb80_sb = fb_sb[0:80, FB_PB80:FB_PB80 + 1]
        trigb_sb = fb_sb[:, FB_TRIG:FB_TRIG + 2]
        idf_sb = fb_sb[:, FB_IDF:FB_IDF + 128]
        mbd_sb = rb_sb[:, RB_MBD:RB_MBD + 128]
        pbd_sb = rb_sb[:, RB_PBD:RB_PBD + 80]
        pre_wt_sb = bb_sb[:, BB_PWT:BB_PWT + 128]
        idb_sb = bb_sb[:, BB_IDB:BB_IDB + 128]

        out2_sb = consts.tile([80, 1024], F32)
        # tanh staging, bf16: [32n rows (32j+f), 512 cols per unit]
        tanh_sb = consts.tile([128, len(UNITS) * 512], BF16)

        # pin the activation table to silu_and_others once: it contains
        # silu+tanh+sin+square+identity, so no further table loads happen.
        # (CoreSim can't evaluate Silu; the sim build substitutes Tanh —
        # the value is unused either way.)
        silu_sb = consts.tile([128, 1], F32)
        pin("scalar", nc.scalar.activation(silu_sb, fb_sb[:, 0:1],
                                           AF.Tanh if sim_compat else AF.Silu))

        # ---- all 16 x transposes up-front on the sync queue (the critical
        # path); group g covers samples 512g..512(g+1)
        xts = []
        for g in range(GROUPS):
            xt = xt_pool.tile([128, 4 * 512], BF16, name="xt", tag="xt")
            # one xbar transpose per 512-sample group with a fully
            # contiguous 512KB DRAM source: out[p, k, b] = x[b, 128k+p]
            # NOTE: all xbar transposes must stay on ONE HWDGE queue —
            # concurrent transpose streams on the SP and ACT rings
            # corrupt data through the shared xbar (measured twice).
            pin("sync", nc.sync.dma_start(
                xt[:, :].rearrange("p (k b) -> p k b", k=4),
                x[512 * g:512 * (g + 1), :],
                transpose=True))
            xts.append(xt)

        ocol = 0
        for u, (g0, n) in enumerate(UNITS):
            rows = 32 * n
            # ---- pre-net: n groups col-tiled onto PE strips 32j.
            # po[32j + f, s] = pre_out feature f of sample 512*(g0+j) + s.
            # psum pending-zero state is per-partition, so each col-strip
            # opens/closes its own accumulation group (start on its k=0,
            # stop on its k=3); skip_group_check silences the bank-granular
            # build-time checker which doesn't model per-strip groups.
            # Strip-major (j outer) order: strip j's 4-matmul chain starts
            # as soon as ITS group's transpose lands, and neighboring
            # strips' chains overlap on the PE (distinct col-groups).
            po = ps_po.tile([128, 512], F32, name="po", tag="po")
            for j in range(n):
                for k in range(4):
                    pin("tensor", nc.tensor.matmul(
                        po[32 * j:32 * (j + 1), :],
                        lhsT=pre_wt_sb[:, 32 * k:32 * k + 32],
                        rhs=xts[g0 + j][:, :].rearrange(
                            "p (k b) -> p k b", k=4)[:, k, :],
                        start=(k == 0), stop=(k == 3),
                        tile_position=(0, 32 * j),
                        skip_group_check=True))
            # fused bias + tanh on the whole unit, bf16 out
            tq = tanh_sb[0:rows, 512 * u:512 * (u + 1)]
            pin("scalar", nc.scalar.activation(tq, po[0:rows, :], AF.Tanh,
                                               bias=fb_sb[0:rows,
                                                          FB_PREB:FB_PREB + 1]))

            # ---- flip to sample-major: 4 PE transposes [32n,128] -> psum.
            # thT[p, 32n*k + 32j + f] = tanh feature f of sample
            # 512*(g0+j) + 128k + p  (cols 32j+4..32j+31 are garbage)
            thT = ps_th.tile([128, 512], BF16, name="thT", tag="thT")
            for k in range(4):
                pin("tensor", nc.tensor.transpose(
                    thT[:, rows * k:rows * (k + 1)],
                    tq[:, 128 * k:128 * (k + 1)], idb_sb[0:rows, 0:rows]))

            # ---- trig: cos/sin of theta = PI4*t + {3pi/4, pi/4}
            # cs[p, (k, j, w, x)]
            cs = work.tile([128, 128], F32, name="cs", tag="cs")
            cs5 = cs[:, 0:32 * n].rearrange("p (k j w x) -> p k j w x",
                                            k=4, j=n, w=4, x=2)
            thT4 = thT[:, 0:4 * rows].rearrange("p (k j w) -> p k j w",
                                                k=4, j=n)
            pin("scalar", nc.scalar.activation(
                cs5[:, :, :, :, 0], thT4[:, :, :, 0:4],
                AF.Sin, bias=trigb_sb[:, 0:1], scale=PI4))
            pin("scalar", nc.scalar.activation(
                cs5[:, :, :, :, 1], thT4[:, :, :, 0:4],
                AF.Sin, bias=trigb_sb[:, 1:2], scale=PI4))

            # ---- psi = v0 (x) v1 (x) v2 (x) v3 per tile kj (kj = k*n+j)
            nt = 4 * n   # sample tiles in this unit
            cs4 = cs[:, 0:32 * n].rearrange("p (kj w x) -> p kj w x",
                                            w=4, x=2)
            v01 = work.tile([128, 64], F32, name="v01", tag="v01")
            v23 = work.tile([128, 64], F32, name="v23", tag="v23")
            pin("vector", nc.vector.tensor_tensor(
                out=v01[:, 0:4 * nt].rearrange("p (t a b) -> p t a b",
                                               a=2, b=2),
                in0=cs4[:, :, 0, :].unsqueeze(3).broadcast_to((128, nt, 2, 2)),
                in1=cs4[:, :, 1, :].unsqueeze(2).broadcast_to((128, nt, 2, 2)),
                op=mybir.AluOpType.mult))
            pin("vector", nc.vector.tensor_tensor(
                out=v23[:, 0:4 * nt].rearrange("p (t a b) -> p t a b",
                                               a=2, b=2),
                in0=cs4[:, :, 2, :].unsqueeze(3).broadcast_to((128, nt, 2, 2)),
                in1=cs4[:, :, 3, :].unsqueeze(2).broadcast_to((128, nt, 2, 2)),
                op=mybir.AluOpType.mult))
            psi = work.tile([128, 256], F32, name="psi", tag="psi")
            pin("vector", nc.vector.tensor_tensor(
                out=psi[:, 0:16 * nt].rearrange("p (t a b) -> p t a b",
                                                a=4, b=4),
                in0=v01[:, 0:4 * nt].rearrange("p (t i) -> p t i", i=4)
                    .unsqueeze(3).broadcast_to((128, nt, 4, 4)),
                in1=v23[:, 0:4 * nt].rearrange("p (t i) -> p t i", i=4)
                    .unsqueeze(2).broadcast_to((128, nt, 4, 4)),
                op=mybir.AluOpType.mult))

            # ---- quantum circuit per slab of 8 tiles (nt/8 slabs), all
            # slabs of the unit share one 128*ns-col M and P matmul
            # (float32r: full-rate rows at N>=256, ~fp32 accuracy)
            ns = nt // 8
            psiT = work.tile([128, 256], F32R, name="psiT", tag="psiT")
            for h in range(ns):
                psiT_ps = ps_ct.tile([128, 128], F32, name="psiT_ps", tag="pT")
                pin("tensor", nc.tensor.transpose(
                    psiT_ps, psi[:, 128 * h:128 * (h + 1)], idf_sb))
                pin("vector", nc.vector.tensor_copy(
                    psiT[:, 128 * h:128 * (h + 1)], psiT_ps))
            phiT_ps = ps_ct.tile([128, 256], F32, name="phiT_ps", tag="phT")
            pin("tensor", nc.tensor.matmul(
                phiT_ps[:, 0:128 * ns], lhsT=mbd_sb, rhs=psiT[:, 0:128 * ns],
                start=True, stop=True))
            phi2 = work.tile([128, 256], F32R, name="phi2", tag="phi2")
            pin("scalar", nc.scalar.activation(
                phi2[:, 0:128 * ns], phiT_ps[:, 0:128 * ns], AF.Square))
            o10_ps = ps_ct.tile([80, 256], F32, name="o10_ps", tag="o10")
            pin("tensor", nc.tensor.matmul(
                o10_ps[:, 0:128 * ns], lhsT=pbd_sb, rhs=phi2[:, 0:128 * ns],
                start=True, stop=True))
            # bias-add into the output staging tile, then store this unit's
            # slice right away (plain copy on the ACT ring; overlaps the
            # SP-ring transpose stream)
            pin("scalar", nc.scalar.activation(
                out2_sb[:, ocol:ocol + 128 * ns],
                o10_ps[:, 0:128 * ns], AF.Identity, bias=pb80_sb))
            pin("scalar", nc.scalar.dma_start(
                out[:, ocol:ocol + 128 * ns],
                out2_sb[:, ocol:ocol + 128 * ns]))
            ocol += 128 * ns

    nc.finalize()  # bacc: register alloc + event-semaphore wait splitting
    return nc


_NC_CACHE: dict = {}


def _get_nc() -> bass.Bass:
    if "nc" not in _NC_CACHE:
        _NC_CACHE["nc"] = build_nc()
    return _NC_CACHE["nc"]


def make_in_maps(inputs: dict) -> list:
    x = np.asarray(inputs["input_features"], np.float32)
    pre_w = np.asarray(inputs["pre_w"], np.float32)
    pre_b = np.asarray(inputs["pre_b"], np.float32)
    q_params = np.asarray(inputs["q_params"], np.float32)
    post_w = np.asarray(inputs["post_w"], np.float32)
    post_b = np.asarray(inputs["post_b"], np.float32)

    M = _build_M(q_params)
    P = _build_P(post_w)
    rblob = np.zeros((128, RB_COLS), np.float32)
    for t in range(8):
        rblob[16 * t:16 * (t + 1), RB_MBD + 16 * t:RB_MBD + 16 * (t + 1)] = M.T
        rblob[16 * t:16 * (t + 1), RB_PBD + 10 * t:RB_PBD + 10 * (t + 1)] = P

    fblob = np.zeros((128, FB_COLS), np.float32)
    for j in range(4):
        fblob[32 * j:32 * j + 4, FB_PREB] = pre_b
    fblob[0:80, FB_PB80] = np.tile(post_b, 8)
    fblob[:, FB_TRIG + 0] = 3.0 * PI4
    fblob[:, FB_TRIG + 1] = PI4
    fblob[:, FB_IDF:FB_IDF + 128] = np.eye(128, dtype=np.float32)

    bblob = np.zeros((128, BB_COLS), np.float32)
    # pre_wt[p, 32k + f] = pre_w[f, 128k+p], zero-padded to 32 cols/chunk
    for k in range(4):
        bblob[:, BB_PWT + 32 * k:BB_PWT + 32 * k + 4] = \
            pre_w[:, 128 * k:128 * (k + 1)].T
    bblob[:, BB_IDB:BB_IDB + 128] = np.eye(128, dtype=np.float32)
    bblob = bblob.astype(ml_dtypes.bfloat16)

    xb = x.astype(ml_dtypes.bfloat16)
    consts = dict(fblob=fblob, rblob=rblob, bblob=bblob)
    return [dict(x=xb[B * i:B * (i + 1)], **consts) for i in range(N_CORES)]


def _out_index() -> np.ndarray:
    """Map device out [80, 1024] -> sample/class gather indices.

    Device col 128*s + p (s = global slab) and partition 10*m + c hold
    class c of sample 512*(g0+j) + 128k + p, where within the slab's
    unit kj = 8*h + m (h = slab index within the unit), k = kj//n,
    j = kj%n.
    """
    idx_p = np.zeros((B, C), np.int64)
    idx_c = np.zeros((B, C), np.int64)
    s = 0
    for (g0, n) in UNITS:
        for h in range(n // 2):
            for m in range(8):
                kj = 8 * h + m
                k, j = kj // n, kj % n
                base = 512 * (g0 + j) + 128 * k
                samples = base + np.arange(128)
                for c in range(C):
                    idx_p[samples, c] = 10 * m + c
                    idx_c[samples, c] = 128 * s + np.arange(128)
            s += 1
    return np.stack([idx_p, idx_c], axis=-1)


_OUT_IDX = _out_index()


def unpack_out(dev_out: np.ndarray) -> np.ndarray:
    """[80, 1024] device layout -> [B, C]."""
    return np.ascontiguousarray(dev_out[_OUT_IDX[..., 0], _OUT_IDX[..., 1]])


def run_on_device(inputs: dict, **kwargs):
    """Returns (full_output, BassKernelResults)."""
    nc = _get_nc()
    in_maps = make_in_maps(inputs)
    res = run_bass_kernel_spmd(nc, in_maps, core_ids=list(range(N_CORES)),
                               **kwargs)
    full = np.concatenate(
        [unpack_out(res.results[i]["out"]) for i in range(N_CORES)], 0)
    return np.ascontiguousarray(full, dtype=np.float32), res


def kernel(**inputs) -> np.ndarray:
    out, _ = run_on_device(inputs)
    return out
